# revision 32
# baseline (speedup 1.0000x reference)
"""Trainium2 Bass kernel for a 6-layer transformer encoder (B=8, S=512,
D=1024, H=16, DFF=512), data-parallel over batch across 8 NeuronCores.

Per-core layout strategy:
  - residual stream kept transposed: xT [D, S] f32r (channels on partitions)
  - Q^T, K^T computed transposed (bf16); V computed natural [S, D] (bf16)
  - scores computed in BOTH layouts: q-major for the HBM attn output
    (softmax rowsums free via activation accum_out) and k-major (bf16)
    for the attn@V matmul.  Softmax without max-subtraction (scores are
    tightly bounded for this problem's 0.02-scaled weights).
  - attn@V normalization deferred: OT_unnorm scaled by 1/rowsum, with the
    per-token reciprocals broadcast across partitions via rank-1 (K=1)
    matmuls; rowsums along partitions via ones-vector matmuls.
  - LayerNorm on the transposed stream: token mean/var via ones-matmuls,
    per-token scale/shift built as rank-1 products alpha (x) rstd.
  - float32r (full-rate fp32 matmul mode) for residual-stream GEMMs;
    bf16 for the attention score/probability path.
"""

import sys
import numpy as np

B, S, D, H, NL, DFF = 8, 512, 1024, 16, 6, 512
DK = D // H  # 64
P = 128
NM = D // P    # 8
NS = S // P    # 4
NF = DFF // P  # 4
EPS = 1e-6

_CACHE = {}


def _build(nl=NL):
    sys.path.insert(0, "/opt/trn_rl_repo")
    import concourse.mybir as mybir
    import concourse.tile as tile
    from concourse import bacc
    from concourse.masks import make_identity
    from contextlib import ExitStack

    dt = mybir.dt
    F32, F32R, BF16 = dt.float32, dt.float32r, dt.bfloat16
    AF = mybir.ActivationFunctionType

    nc = bacc.Bacc("TRN2", target_bir_lowering=False, debug=False)

    src = nc.dram_tensor("src", [S, D], F32, kind="ExternalInput").ap()
    Wq = nc.dram_tensor("Wq", [nl, D, D], F32R, kind="ExternalInput").ap()
    Wk = nc.dram_tensor("Wk", [nl, D, D], F32R, kind="ExternalInput").ap()
    Wv = nc.dram_tensor("Wv", [nl, D, D], F32R, kind="ExternalInput").ap()
    Wo = nc.dram_tensor("Wo", [nl, D, D], F32R, kind="ExternalInput").ap()
    W1 = nc.dram_tensor("W1", [nl, D, DFF], F32R, kind="ExternalInput").ap()
    W2 = nc.dram_tensor("W2", [nl, DFF, D], F32R, kind="ExternalInput").ap()
    bq = nc.dram_tensor("bq", [nl, D], F32, kind="ExternalInput").ap()
    bk = nc.dram_tensor("bk", [nl, D], F32, kind="ExternalInput").ap()
    bv = nc.dram_tensor("bv", [nl, D], F32, kind="ExternalInput").ap()
    bo = nc.dram_tensor("bo", [nl, D], F32, kind="ExternalInput").ap()
    b1 = nc.dram_tensor("b1", [nl, DFF], F32, kind="ExternalInput").ap()
    b2 = nc.dram_tensor("b2", [nl, D], F32, kind="ExternalInput").ap()
    ln1_a = nc.dram_tensor("ln1_a", [nl, D], F32R, kind="ExternalInput").ap()
    ln1_b = nc.dram_tensor("ln1_b", [nl, D], F32R, kind="ExternalInput").ap()
    ln2_a = nc.dram_tensor("ln2_a", [nl, D], F32R, kind="ExternalInput").ap()
    ln2_b = nc.dram_tensor("ln2_b", [nl, D], F32R, kind="ExternalInput").ap()
    out_x = nc.dram_tensor("out_x", [S, D], F32, kind="ExternalOutput").ap()
    out_attn = nc.dram_tensor("out_attn", [nl, H, S, S], BF16,
                              kind="ExternalOutput").ap()

    with tile.TileContext(nc) as tc, ExitStack() as ctx:
        sbp = ctx.enter_context(tc.tile_pool(name="sbp", bufs=1))
        pl = ctx.enter_context(tc.tile_pool(name="pl", bufs=1))  # per-tile bufs
        # PSUM pools (8 banks total: 4 + 2 + 2)
        psm = ctx.enter_context(tc.tile_pool(name="psm", bufs=2, space="PSUM"))
        pss = ctx.enter_context(tc.tile_pool(name="pss", bufs=2, space="PSUM"))
        psT = ctx.enter_context(tc.tile_pool(name="psT", bufs=1, space="PSUM"))
        pso = ctx.enter_context(tc.tile_pool(name="pso", bufs=2, space="PSUM"))

        ident = sbp.tile([P, P], F32, tag="ident")
        make_identity(nc, ident[:])
        ones_stage = sbp.tile([P, S], F32, tag="ones_stage")
        nc.vector.memset(ones_stage[:], 1.0)
        ones_col_b = sbp.tile([P, 1], BF16, tag="ones_col_b")
        nc.vector.tensor_copy(ones_col_b[:], ones_stage[:, 0:1])
        ones_col_r = sbp.tile([P, 1], F32R, tag="ones_col_r")
        nc.vector.tensor_copy(ones_col_r[:], ones_stage[:, 0:1])
        ones64 = sbp.tile([P, DK], BF16, tag="ones64")
        nc.vector.tensor_copy(ones64[:], ones_stage[:, 0:DK])
        ones_row = sbp.tile([1, S], F32R, tag="ones_row")
        nc.vector.tensor_copy(ones_row[:], ones_stage[0:1, :])
        eps_c = sbp.tile([1, 1], F32, tag="eps_c")
        nc.vector.memset(eps_c[:], float(D * D * EPS))

        def mk(shape, dtp, tag, bufs, name):
            return pl.tile(shape, dtp, tag=tag, bufs=bufs, name=name)

        # ---- initial transpose: src [S,D] -> xT (8 tiles [128,S], f32r) ----
        xT = [mk([P, S], F32R, "x", 9, f"x_init{m}") for m in range(NM)]
        for st in range(NS):
            t = mk([P, D], F32, "t", 2, f"xnat{st}")
            nc.sync.dma_start(t[:], src[st * P:(st + 1) * P, :])
            for m in range(NM):
                pt = psm.tile([P, S], F32, tag="m", name=f"tp{st}_{m}")
                nc.tensor.transpose(pt[:, 0:P], t[:, m * P:(m + 1) * P], ident[:])
                nc.vector.tensor_copy(xT[m][:, st * P:(st + 1) * P], pt[:, 0:P])

        def load_cols(vec_ap, n, tag, name):
            nat = mk([n, P], F32, "natc", 2, name + "n")
            nc.sync.dma_start(nat[:], vec_ap.rearrange("(j p) -> j p", p=P))
            pt = psm.tile([P, S], F32, tag="m", name=name + "p")
            nc.tensor.transpose(pt[:, 0:n], nat[:], ident[0:n, 0:n])
            col = mk([P, NM], F32, tag, 2, name)
            nc.vector.tensor_copy(col[:, 0:n], pt[:, 0:n])
            return col

        def layer_norm(l, i, y, ar, br):
            """y: NM tiles [P,S] f32r holding (x + sublayer out), consumed.
            Returns NM new residual tiles LN(y)."""
            st_ps = psm.tile([P, S], F32, tag="m", name=f"st{l}_{i}")
            for m in range(NM):
                nc.tensor.matmul(st_ps[0:1, :], ones_col_r[:], y[m][:],
                                 start=(m == 0), stop=(m == NM - 1))
            st2_ps = psm.tile([P, S], F32, tag="m", name=f"st2{l}_{i}")
            for m in range(NM):
                yq = mk([P, S], F32R, "ysq", 2, f"ysq{l}_{i}_{m}")
                nc.gpsimd.tensor_mul(yq[:], y[m][:], y[m][:])
                nc.tensor.matmul(st2_ps[0:1, :], ones_col_r[:], yq[:],
                                 start=(m == 0), stop=(m == NM - 1))
            s1t = mk([1, S], F32, "tiny", 4, f"s1_{l}_{i}")
            nc.vector.tensor_copy(s1t[:], st_ps[0:1, :])
            sst = mk([1, S], F32, "tiny", 4, f"ss_{l}_{i}")
            nc.vector.tensor_copy(sst[:], st2_ps[0:1, :])
            s1 = s1t[:]
            ss = sst[:]
            u = mk([1, S], F32, "tiny", 4, f"u{l}_{i}")
            nc.vector.tensor_mul(u[:], s1, s1)
            t1 = mk([1, S], F32, "tiny", 4, f"t1{l}_{i}")
            nc.vector.tensor_scalar_mul(t1[:], ss, float(D))
            w = mk([1, S], F32, "tiny", 4, f"w{l}_{i}")
            nc.vector.tensor_sub(w[:], t1[:], u[:])
            # rstd = D / sqrt(w + C) via Exp(-0.5 * Log(w + C)): stays on the
            # exp ACT table set (no table swap) and avoids the slow
            # single-partition DVE reciprocal.
            lg = mk([1, S], F32, "tiny", 4, f"lg{l}_{i}")
            nc.scalar.activation(lg[:], w[:], AF.Ln, bias=eps_c[:])
            r = mk([1, S], F32, "tiny", 4, f"r{l}_{i}")
            nc.scalar.activation(r[:], lg[:], AF.Exp, scale=-0.5)
            rstd = mk([1, S], F32R, "tiny", 4, f"rstd{l}_{i}")
            nc.vector.tensor_scalar_mul(rstd[:], r[:], float(D))
            s1rn = mk([1, S], F32R, "tiny", 4, f"s1rn{l}_{i}")
            nc.vector.tensor_mul(s1rn[:], s1, r[:])
            nc.vector.tensor_scalar_mul(s1rn[:], s1rn[:], -1.0)
            x_new = []
            for m in range(NM):
                sc_ps = psm.tile([P, S], F32, tag="m", name=f"sc{l}_{i}_{m}")
                nc.tensor.matmul(sc_ps[:], ar[0:1, m * P:(m + 1) * P],
                                 rstd[:], start=True, stop=True)
                sh_ps = psm.tile([P, S], F32, tag="m", name=f"sh{l}_{i}_{m}")
                nc.tensor.matmul(sh_ps[:], ar[0:1, m * P:(m + 1) * P],
                                 s1rn[:], start=True, stop=False)
                nc.tensor.matmul(sh_ps[:], br[0:1, m * P:(m + 1) * P],
                                 ones_row[:], start=False, stop=True)
                nc.vector.tensor_mul(y[m][:], y[m][:], sc_ps[:])
                xt = mk([P, S], F32R, "x", 9, f"x{l}_{i}_{m}")
                nc.vector.tensor_add(xt[:], y[m][:], sh_ps[:])
                x_new.append(xt)
            return x_new

        for l in range(nl):
            bq_c = load_cols(bq[l], NM, "bqc", f"bqc{l}")
            bk_c = load_cols(bk[l], NM, "bkc", f"bkc{l}")
            bv_c = load_cols(bv[l], NM, "bvc", f"bvc{l}")
            bo_c = load_cols(bo[l], NM, "boc", f"boc{l}")
            b1_c = load_cols(b1[l], NF, "b1c", f"b1c{l}")
            b2_c = load_cols(b2[l], NM, "b2c", f"b2c{l}")
            lrows = []
            for i, (lna, lnb) in enumerate(((ln1_a, ln1_b), (ln2_a, ln2_b))):
                ar = mk([1, D], F32R, "lnrow", 3, f"ar{l}_{i}")
                nc.sync.dma_start(ar[:], lna[l][None, :])
                br = mk([1, D], F32R, "lnrow", 3, f"br{l}_{i}")
                nc.sync.dma_start(br[:], lnb[l][None, :])
                lrows.append((ar, br))

            wq_t = [mk([P, D], F32R, "w", 10, f"wq{l}_{i}") for i in range(NM)]
            wk_t = [mk([P, D], F32R, "w", 10, f"wk{l}_{i}") for i in range(NM)]
            wv_t = [mk([P, D], F32R, "w", 10, f"wv{l}_{i}") for i in range(NM)]
            for wt, W in ((wq_t, Wq), (wk_t, Wk), (wv_t, Wv)):
                for k in range(NM):
                    nc.sync.dma_start(wt[k][:], W[l, k * P:(k + 1) * P, :])

            # ---- QT / KT (bf16) ----
            QT, KT = [], []
            for (dst, wt, bcol, tg) in ((QT, wq_t, bq_c, "q"), (KT, wk_t, bk_c, "k")):
                for m in range(NM):
                    pt = psm.tile([P, S], F32, tag="m", name=f"{tg}p{l}_{m}")
                    for k in range(NM):
                        nc.tensor.matmul(pt[:], wt[k][:, m * P:(m + 1) * P],
                                         xT[k][:], start=(k == 0),
                                         stop=(k == NM - 1))
                    t = mk([P, S], BF16, tg, 9, f"{tg}{l}_{m}")
                    nc.vector.tensor_scalar_add(t[:], pt[:], bcol[:, m:m + 1])
                    dst.append(t)

            # ---- V natural [S, D] bf16 ----
            Vt = []
            for stt in range(NS):
                t = mk([P, D], BF16, "v", 5, f"v{l}_{stt}")
                for nch in range(2):
                    pt = psm.tile([P, S], F32, tag="m", name=f"vp{l}_{stt}_{nch}")
                    for k in range(NM):
                        nc.tensor.matmul(
                            pt[:], xT[k][:, stt * P:(stt + 1) * P],
                            wv_t[k][:, nch * 512:(nch + 1) * 512],
                            start=(k == 0), stop=(k == NM - 1))
                    nc.scalar.activation(t[:, nch * 512:(nch + 1) * 512], pt[:],
                                         AF.Identity)
                Vt.append(t)

            wo_t = [mk([P, D], F32R, "w", 10, f"wo{l}_{i}") for i in range(NM)]
            for k in range(NM):
                nc.sync.dma_start(wo_t[k][:], Wo[l, k * P:(k + 1) * P, :])
            w1_t = [mk([P, DFF], F32R, "w1", 9, f"w1{l}_{i}") for i in range(NM)]
            for k in range(NM):
                nc.sync.dma_start(w1_t[k][:], W1[l, k * P:(k + 1) * P, :])
            w2_t = [mk([P, D], F32R, "w", 10, f"w2{l}_{i}") for i in range(NF)]
            for k in range(NF):
                nc.sync.dma_start(w2_t[k][:], W2[l, k * P:(k + 1) * P, :])

            # ---- attention (processed in head pairs) ----
            # rowsum reciprocals come from the q-major exp's accum_out,
            # stored sparsely at free columns {0,32,64,96} so a PE transpose
            # lands them on 32-aligned partitions for the K=1 broadcast
            # matmuls (f32r can't use tile_position; the broadcast runs bf16).
            OTn = []
            for pr in range(8):
                ot_ps = pso.tile([P, S], F32, tag="o", name=f"otp{l}_{pr}")
                rbs = []
                for hh in range(2):
                    h = 2 * pr + hh
                    ho = (h % 2) * DK
                    q_hT = QT[pr][ho:ho + DK, :]
                    k_hT = KT[pr][ho:ho + DK, :]

                    rs = mk([P, NS], F32, "rs", 2, f"rs{l}_{h}")
                    atn = []
                    for qt in range(NS):
                        pt = pss.tile([P, S], F32, tag="s", name=f"sc{l}_{h}_{qt}")
                        nc.tensor.matmul(pt[:], q_hT[:, qt * P:(qt + 1) * P],
                                         k_hT[:], start=True, stop=True)
                        a = mk([P, S], BF16, "attn", 5, f"at{l}_{h}_{qt}")
                        nc.scalar.activation(a[:], pt[:], AF.Exp, scale=0.125)
                        nc.vector.tensor_reduce(rs[:, qt:qt + 1], a[:],
                                                mybir.AxisListType.X,
                                                mybir.AluOpType.add)
                        atn.append(a)
                    et = []
                    eth = [mk([P, 2 * S], BF16, "et", 2, f"et{l}_{h}_{half}")
                           for half in range(2)]
                    for half in range(2):
                        pt = psT.tile([P, 2 * S], F32, tag="T", name=f"sT{l}_{h}_{half}")
                        for j in range(2):
                            kt = 2 * half + j
                            nc.tensor.matmul(pt[:, j * S:(j + 1) * S],
                                             k_hT[:, kt * P:(kt + 1) * P],
                                             q_hT[:], start=True, stop=True)
                        nc.scalar.activation(eth[half][:], pt[:], AF.Exp,
                                             scale=0.125)
                        et.append(eth[half][:, 0:S])
                        et.append(eth[half][:, S:2 * S])
                    for kt in range(NS):
                        nc.tensor.matmul(ot_ps[ho:ho + DK, :],
                                         Vt[kt][:, h * DK:(h + 1) * DK],
                                         et[kt][:],
                                         start=(kt == 0), stop=(kt == NS - 1),
                                         tile_position=(0, ho))

                    rcp = mk([P, NS], F32, "rcp", 2, f"rcp{l}_{h}")
                    nc.vector.reciprocal(rcp[:], rs[:])
                    for qt in range(NS):
                        nc.vector.tensor_scalar_mul(atn[qt][:], atn[qt][:],
                                                    rcp[:, qt:qt + 1])
                        nc.sync.dma_start(
                            out_attn[l, h, qt * P:(qt + 1) * P, :], atn[qt][:])
                    # reciprocals -> one [1,512] row (PE transposes), then
                    # broadcast across partitions on the (idle) GpSimd
                    rcpt_ps = psm.tile([P, S], F32, tag="m", name=f"rtp{l}_{h}")
                    for qt in range(NS):
                        nc.tensor.transpose(
                            rcpt_ps[0:1, qt * P:(qt + 1) * P],
                            rcp[:, qt:qt + 1], ident[:])
                    rrow = mk([1, S], F32, "rrow", 2, f"rrow{l}_{h}")
                    nc.scalar.activation(rrow[:], rcpt_ps[0:1, :], AF.Identity)
                    rb_h = mk([P, S], F32, "rb", 3, f"rbs{l}_{h}")
                    nc.gpsimd.partition_broadcast(rb_h[:], rrow[:])
                    rbs.append(rb_h)
                # pair epilogue: OT_norm = OT_unnorm * recipB + bv
                ot = mk([P, S], F32R, "ot", 9, f"ot{l}_{pr}")
                nc.vector.tensor_mul(ot[0:DK, :], ot_ps[0:DK, :],
                                     rbs[0][0:DK, :])
                nc.vector.tensor_mul(ot[DK:P, :], ot_ps[DK:P, :],
                                     rbs[1][DK:P, :])
                nc.vector.tensor_scalar_add(ot[:], ot[:], bv_c[:, pr:pr + 1])
                OTn.append(ot)

            # ---- O projection (+bo) -> y -> LN1 ----
            y1 = []
            for m in range(NM):
                pt = psm.tile([P, S], F32, tag="m", name=f"op{l}_{m}")
                for k in range(NM):
                    nc.tensor.matmul(pt[:], wo_t[k][:, m * P:(m + 1) * P],
                                     OTn[k][:], start=(k == 0),
                                     stop=(k == NM - 1))
                yt = mk([P, S], F32R, "y", 9, f"y1_{l}_{m}")
                nc.scalar.activation(yt[:], pt[:], AF.Identity,
                                     bias=bo_c[:, m:m + 1])
                nc.vector.tensor_add(yt[:], yt[:], xT[m][:])
                y1.append(yt)
            ar, br = lrows[0]
            xT = layer_norm(l, 0, y1, ar, br)

            # ---- FFN ----
            hT = []
            for mf in range(NF):
                pt = psm.tile([P, S], F32, tag="m", name=f"f1p{l}_{mf}")
                for k in range(NM):
                    nc.tensor.matmul(pt[:], w1_t[k][:, mf * P:(mf + 1) * P],
                                     xT[k][:], start=(k == 0),
                                     stop=(k == NM - 1))
                t = mk([P, S], F32R, "h", 5, f"h{l}_{mf}")
                nc.scalar.activation(t[:], pt[:], AF.Relu,
                                     bias=b1_c[:, mf:mf + 1])
                hT.append(t)
            y2 = []
            for m in range(NM):
                pt = psm.tile([P, S], F32, tag="m", name=f"f2p{l}_{m}")
                for kt in range(NF):
                    nc.tensor.matmul(pt[:], w2_t[kt][:, m * P:(m + 1) * P],
                                     hT[kt][:], start=(kt == 0),
                                     stop=(kt == NF - 1))
                yt = mk([P, S], F32R, "y", 9, f"y2_{l}_{m}")
                nc.scalar.activation(yt[:], pt[:], AF.Identity,
                                     bias=b2_c[:, m:m + 1])
                nc.vector.tensor_add(yt[:], yt[:], xT[m][:])
                y2.append(yt)
            ar, br = lrows[1]
            xT = layer_norm(l, 1, y2, ar, br)

        # ---- final transpose back: xT -> out_x [S, D] ----
        for stt in range(NS):
            t = mk([P, D], F32, "t", 2, f"xout{stt}")
            for m in range(NM):
                pt = psm.tile([P, S], F32, tag="m", name=f"tpf{stt}_{m}")
                nc.tensor.transpose(
                    pt[:, 0:P],
                    xT[m][:, stt * P:(stt + 1) * P].bitcast(F32), ident[:])
                nc.vector.tensor_copy(t[:, m * P:(m + 1) * P], pt[:, 0:P])
            nc.sync.dma_start(out_x[stt * P:(stt + 1) * P, :], t[:])

    nc.compile()
    return nc


def _get_nc(nl=NL):
    key = ("nc", nl)
    if key not in _CACHE:
        _CACHE[key] = _build(nl)
    return _CACHE[key]


def kernel(src, Wq, bq, Wk, bk, Wv, bv, Wo, bo,
           ln1_a, ln1_b, ln2_a, ln2_b, W1, b1, W2, b2, nl=NL, trace=False):
    sys.path.insert(0, "/opt/trn_rl_repo")
    from concourse.bass_utils import run_bass_kernel_spmd

    nc = _get_nc(nl)
    if trace:
        try:
            import types
            import antenv
            if 'antenv.axon_hooks' not in sys.modules:
                mod = types.ModuleType('antenv.axon_hooks')
                _h = {}
                mod.set_axon_ntff_profile_hook = lambda h: _h.__setitem__('h', h)
                mod.get_axon_ntff_profile_hook = lambda: _h.get('h')
                sys.modules['antenv.axon_hooks'] = mod
                antenv.axon_hooks = mod
                if '/root/.axon_site' not in sys.path:
                    sys.path.insert(0, '/root/.axon_site')
                from trn_agent_boot.trn_boot import _ntff_profile_via_ctypes
                mod.set_axon_ntff_profile_hook(
                    _ntff_profile_via_ctypes('/opt/axon/libaxon_pjrt.so'))
            import concourse.bass_utils as bu
            bu.upload_artifacts = lambda tmpdir: f"local:{tmpdir}"
        except Exception as e:
            print("trace setup failed:", repr(e))
    f32 = np.float32
    shared = dict(
        Wq=np.ascontiguousarray(Wq, f32), Wk=np.ascontiguousarray(Wk, f32),
        Wv=np.ascontiguousarray(Wv, f32), Wo=np.ascontiguousarray(Wo, f32),
        W1=np.ascontiguousarray(W1, f32), W2=np.ascontiguousarray(W2, f32),
        bq=np.ascontiguousarray(bq, f32), bk=np.ascontiguousarray(bk, f32),
        bv=np.ascontiguousarray(bv, f32), bo=np.ascontiguousarray(bo, f32),
        b1=np.ascontiguousarray(b1, f32), b2=np.ascontiguousarray(b2, f32),
        ln1_a=np.ascontiguousarray(ln1_a, f32),
        ln1_b=np.ascontiguousarray(ln1_b, f32),
        ln2_a=np.ascontiguousarray(ln2_a, f32),
        ln2_b=np.ascontiguousarray(ln2_b, f32),
    )
    in_maps = []
    for b in range(B):
        m = dict(shared)
        m["src"] = np.ascontiguousarray(src[b], f32)
        in_maps.append(m)
    res = run_bass_kernel_spmd(nc, in_maps, core_ids=list(range(B)),
                               trace=trace,
                               tmpdir="/tmp/ktrace" if trace else None)
    x = np.stack([res.results[b]["out_x"] for b in range(B)])
    attn = np.stack([res.results[b]["out_attn"].astype(np.float32)
                     for b in range(B)])
    if trace:
        _CACHE["exec_time_ns"] = res.exec_time_ns
    return (x, attn)


# revision 33
# speedup vs baseline: 1.2874x; 1.2874x over previous
"""Trainium2 Bass kernel for a 6-layer transformer encoder (B=8, S=512,
D=1024, H=16, DFF=512), data-parallel over batch across 8 NeuronCores.

Per-core layout strategy:
  - residual stream kept transposed: xT [D, S] f32r (channels on partitions)
  - Q^T, K^T computed transposed (bf16); V computed natural [S, D] (bf16)
  - scores computed in BOTH layouts: q-major for the HBM attn output
    (softmax rowsums free via activation accum_out) and k-major (bf16)
    for the attn@V matmul.  Softmax without max-subtraction (scores are
    tightly bounded for this problem's 0.02-scaled weights).
  - attn@V normalization deferred: OT_unnorm scaled by 1/rowsum, with the
    per-token reciprocals broadcast across partitions via rank-1 (K=1)
    matmuls; rowsums along partitions via ones-vector matmuls.
  - LayerNorm on the transposed stream: token mean/var via ones-matmuls,
    per-token scale/shift built as rank-1 products alpha (x) rstd.
  - float32r (full-rate fp32 matmul mode) for residual-stream GEMMs;
    bf16 for the attention score/probability path.
"""

import sys
import numpy as np

B, S, D, H, NL, DFF = 8, 512, 1024, 16, 6, 512
DK = D // H  # 64
P = 128
NM = D // P    # 8
NS = S // P    # 4
NF = DFF // P  # 4
EPS = 1e-6

_CACHE = {}


def _build(nl=NL):
    sys.path.insert(0, "/opt/trn_rl_repo")
    import concourse.mybir as mybir
    import concourse.tile as tile
    from concourse import bacc
    from concourse.masks import make_identity
    from contextlib import ExitStack

    dt = mybir.dt
    F32, F32R, BF16 = dt.float32, dt.float32r, dt.bfloat16
    AF = mybir.ActivationFunctionType

    nc = bacc.Bacc("TRN2", target_bir_lowering=False, debug=False)

    src = nc.dram_tensor("src", [S, D], F32, kind="ExternalInput").ap()
    Wq = nc.dram_tensor("Wq", [nl, D, D], F32R, kind="ExternalInput").ap()
    Wk = nc.dram_tensor("Wk", [nl, D, D], F32R, kind="ExternalInput").ap()
    Wv = nc.dram_tensor("Wv", [nl, D, D], F32R, kind="ExternalInput").ap()
    Wo = nc.dram_tensor("Wo", [nl, D, D], F32R, kind="ExternalInput").ap()
    W1 = nc.dram_tensor("W1", [nl, D, DFF], F32R, kind="ExternalInput").ap()
    W2 = nc.dram_tensor("W2", [nl, DFF, D], F32R, kind="ExternalInput").ap()
    bq = nc.dram_tensor("bq", [nl, D], F32, kind="ExternalInput").ap()
    bk = nc.dram_tensor("bk", [nl, D], F32, kind="ExternalInput").ap()
    bv = nc.dram_tensor("bv", [nl, D], F32, kind="ExternalInput").ap()
    bo = nc.dram_tensor("bo", [nl, D], F32, kind="ExternalInput").ap()
    b1 = nc.dram_tensor("b1", [nl, DFF], F32, kind="ExternalInput").ap()
    b2 = nc.dram_tensor("b2", [nl, D], F32, kind="ExternalInput").ap()
    ln1_a = nc.dram_tensor("ln1_a", [nl, D], F32R, kind="ExternalInput").ap()
    ln1_b = nc.dram_tensor("ln1_b", [nl, D], F32R, kind="ExternalInput").ap()
    ln2_a = nc.dram_tensor("ln2_a", [nl, D], F32R, kind="ExternalInput").ap()
    ln2_b = nc.dram_tensor("ln2_b", [nl, D], F32R, kind="ExternalInput").ap()
    out_x = nc.dram_tensor("out_x", [S, D], F32, kind="ExternalOutput").ap()
    out_attn = nc.dram_tensor("out_attn", [nl, H, S, S], BF16,
                              kind="ExternalOutput").ap()

    with tile.TileContext(nc) as tc, ExitStack() as ctx:
        sbp = ctx.enter_context(tc.tile_pool(name="sbp", bufs=1))
        pl = ctx.enter_context(tc.tile_pool(name="pl", bufs=1))  # per-tile bufs
        # PSUM pools (8 banks total: 4 + 2 + 2)
        psm = ctx.enter_context(tc.tile_pool(name="psm", bufs=2, space="PSUM"))
        pss = ctx.enter_context(tc.tile_pool(name="pss", bufs=2, space="PSUM"))
        psT = ctx.enter_context(tc.tile_pool(name="psT", bufs=1, space="PSUM"))
        pso = ctx.enter_context(tc.tile_pool(name="pso", bufs=2, space="PSUM"))

        ident = sbp.tile([P, P], F32, tag="ident")
        make_identity(nc, ident[:])
        ones_stage = sbp.tile([P, S], F32, tag="ones_stage")
        nc.vector.memset(ones_stage[:], 1.0)
        ones_col_b = sbp.tile([P, 1], BF16, tag="ones_col_b")
        nc.vector.tensor_copy(ones_col_b[:], ones_stage[:, 0:1])
        ones_col_r = sbp.tile([P, 1], F32R, tag="ones_col_r")
        nc.vector.tensor_copy(ones_col_r[:], ones_stage[:, 0:1])
        ones64 = sbp.tile([P, DK], BF16, tag="ones64")
        nc.vector.tensor_copy(ones64[:], ones_stage[:, 0:DK])
        ones_row = sbp.tile([1, S], F32R, tag="ones_row")
        nc.vector.tensor_copy(ones_row[:], ones_stage[0:1, :])
        eps_c = sbp.tile([1, 1], F32, tag="eps_c")
        nc.vector.memset(eps_c[:], float(D * D * EPS))

        def mk(shape, dtp, tag, bufs, name):
            return pl.tile(shape, dtp, tag=tag, bufs=bufs, name=name)

        # ---- initial transpose: src [S,D] -> xT (8 tiles [128,S], f32r) ----
        xT = [mk([P, S], F32R, "x", 9, f"x_init{m}") for m in range(NM)]
        for st in range(NS):
            t = mk([P, D], F32, "t", 2, f"xnat{st}")
            nc.sync.dma_start(t[:], src[st * P:(st + 1) * P, :])
            for m in range(NM):
                pt = psm.tile([P, S], F32, tag="m", name=f"tp{st}_{m}")
                nc.tensor.transpose(pt[:, 0:P], t[:, m * P:(m + 1) * P], ident[:])
                nc.vector.tensor_copy(xT[m][:, st * P:(st + 1) * P], pt[:, 0:P])

        def load_cols(vec_ap, n, tag, name):
            nat = mk([n, P], F32, "natc", 2, name + "n")
            nc.sync.dma_start(nat[:], vec_ap.rearrange("(j p) -> j p", p=P))
            pt = psm.tile([P, S], F32, tag="m", name=name + "p")
            nc.tensor.transpose(pt[:, 0:n], nat[:], ident[0:n, 0:n])
            col = mk([P, NM], F32, tag, 2, name)
            nc.vector.tensor_copy(col[:, 0:n], pt[:, 0:n])
            return col

        def layer_norm(l, i, y, ar, br):
            """y: NM tiles [P,S] f32r holding (x + sublayer out), consumed.
            Returns NM new residual tiles LN(y)."""
            st_ps = psm.tile([P, S], F32, tag="m", name=f"st{l}_{i}")
            for m in range(NM):
                nc.tensor.matmul(st_ps[0:1, :], ones_col_r[:], y[m][:],
                                 start=(m == 0), stop=(m == NM - 1))
            st2_ps = psm.tile([P, S], F32, tag="m", name=f"st2{l}_{i}")
            for m in range(NM):
                yq = mk([P, S], F32R, "ysq", 2, f"ysq{l}_{i}_{m}")
                nc.vector.tensor_mul(yq[:], y[m][:], y[m][:])
                nc.tensor.matmul(st2_ps[0:1, :], ones_col_r[:], yq[:],
                                 start=(m == 0), stop=(m == NM - 1))
            s1t = mk([1, S], F32, "tiny", 4, f"s1_{l}_{i}")
            nc.vector.tensor_copy(s1t[:], st_ps[0:1, :])
            sst = mk([1, S], F32, "tiny", 4, f"ss_{l}_{i}")
            nc.vector.tensor_copy(sst[:], st2_ps[0:1, :])
            s1 = s1t[:]
            ss = sst[:]
            u = mk([1, S], F32, "tiny", 4, f"u{l}_{i}")
            nc.vector.tensor_mul(u[:], s1, s1)
            t1 = mk([1, S], F32, "tiny", 4, f"t1{l}_{i}")
            nc.vector.tensor_scalar_mul(t1[:], ss, float(D))
            w = mk([1, S], F32, "tiny", 4, f"w{l}_{i}")
            nc.vector.tensor_sub(w[:], t1[:], u[:])
            # rstd = D / sqrt(w + C) via Exp(-0.5 * Log(w + C)): stays on the
            # exp ACT table set (no table swap) and avoids the slow
            # single-partition DVE reciprocal.
            lg = mk([1, S], F32, "tiny", 4, f"lg{l}_{i}")
            nc.scalar.activation(lg[:], w[:], AF.Ln, bias=eps_c[:])
            r = mk([1, S], F32, "tiny", 4, f"r{l}_{i}")
            nc.scalar.activation(r[:], lg[:], AF.Exp, scale=-0.5)
            rstd = mk([1, S], F32R, "tiny", 4, f"rstd{l}_{i}")
            nc.vector.tensor_scalar_mul(rstd[:], r[:], float(D))
            s1rn = mk([1, S], F32R, "tiny", 4, f"s1rn{l}_{i}")
            nc.vector.tensor_mul(s1rn[:], s1, r[:])
            nc.vector.tensor_scalar_mul(s1rn[:], s1rn[:], -1.0)
            x_new = []
            for m in range(NM):
                sc_ps = psm.tile([P, S], F32, tag="m", name=f"sc{l}_{i}_{m}")
                nc.tensor.matmul(sc_ps[:], ar[0:1, m * P:(m + 1) * P],
                                 rstd[:], start=True, stop=True)
                sh_ps = psm.tile([P, S], F32, tag="m", name=f"sh{l}_{i}_{m}")
                nc.tensor.matmul(sh_ps[:], ar[0:1, m * P:(m + 1) * P],
                                 s1rn[:], start=True, stop=False)
                nc.tensor.matmul(sh_ps[:], br[0:1, m * P:(m + 1) * P],
                                 ones_row[:], start=False, stop=True)
                nc.vector.tensor_mul(y[m][:], y[m][:], sc_ps[:])
                xt = mk([P, S], F32R, "x", 9, f"x{l}_{i}_{m}")
                nc.vector.tensor_add(xt[:], y[m][:], sh_ps[:])
                x_new.append(xt)
            return x_new

        for l in range(nl):
            bq_c = load_cols(bq[l], NM, "bqc", f"bqc{l}")
            bk_c = load_cols(bk[l], NM, "bkc", f"bkc{l}")
            bv_c = load_cols(bv[l], NM, "bvc", f"bvc{l}")
            bo_c = load_cols(bo[l], NM, "boc", f"boc{l}")
            b1_c = load_cols(b1[l], NF, "b1c", f"b1c{l}")
            b2_c = load_cols(b2[l], NM, "b2c", f"b2c{l}")
            lrows = []
            for i, (lna, lnb) in enumerate(((ln1_a, ln1_b), (ln2_a, ln2_b))):
                ar = mk([1, D], F32R, "lnrow", 3, f"ar{l}_{i}")
                nc.sync.dma_start(ar[:], lna[l][None, :])
                br = mk([1, D], F32R, "lnrow", 3, f"br{l}_{i}")
                nc.sync.dma_start(br[:], lnb[l][None, :])
                lrows.append((ar, br))

            wq_t = [mk([P, D], F32R, "w", 10, f"wq{l}_{i}") for i in range(NM)]
            wk_t = [mk([P, D], F32R, "w", 10, f"wk{l}_{i}") for i in range(NM)]
            wv_t = [mk([P, D], F32R, "w", 10, f"wv{l}_{i}") for i in range(NM)]
            for wt, W in ((wq_t, Wq), (wk_t, Wk), (wv_t, Wv)):
                for k in range(NM):
                    nc.sync.dma_start(wt[k][:], W[l, k * P:(k + 1) * P, :])

            # ---- QT / KT (bf16) ----
            QT, KT = [], []
            for (dst, wt, bcol, tg) in ((QT, wq_t, bq_c, "q"), (KT, wk_t, bk_c, "k")):
                for m in range(NM):
                    pt = psm.tile([P, S], F32, tag="m", name=f"{tg}p{l}_{m}")
                    for k in range(NM):
                        nc.tensor.matmul(pt[:], wt[k][:, m * P:(m + 1) * P],
                                         xT[k][:], start=(k == 0),
                                         stop=(k == NM - 1))
                    t = mk([P, S], BF16, tg, 9, f"{tg}{l}_{m}")
                    nc.vector.tensor_scalar_add(t[:], pt[:], bcol[:, m:m + 1])
                    dst.append(t)

            # ---- V natural [S, D] bf16 ----
            Vt = []
            for stt in range(NS):
                t = mk([P, D], BF16, "v", 5, f"v{l}_{stt}")
                for nch in range(2):
                    pt = psm.tile([P, S], F32, tag="m", name=f"vp{l}_{stt}_{nch}")
                    for k in range(NM):
                        nc.tensor.matmul(
                            pt[:], xT[k][:, stt * P:(stt + 1) * P],
                            wv_t[k][:, nch * 512:(nch + 1) * 512],
                            start=(k == 0), stop=(k == NM - 1))
                    nc.scalar.activation(t[:, nch * 512:(nch + 1) * 512], pt[:],
                                         AF.Identity)
                Vt.append(t)

            wo_t = [mk([P, D], F32R, "w", 10, f"wo{l}_{i}") for i in range(NM)]
            for k in range(NM):
                nc.sync.dma_start(wo_t[k][:], Wo[l, k * P:(k + 1) * P, :])
            w1_t = [mk([P, DFF], F32R, "w1", 9, f"w1{l}_{i}") for i in range(NM)]
            for k in range(NM):
                nc.sync.dma_start(w1_t[k][:], W1[l, k * P:(k + 1) * P, :])
            w2_t = [mk([P, D], F32R, "w", 10, f"w2{l}_{i}") for i in range(NF)]
            for k in range(NF):
                nc.sync.dma_start(w2_t[k][:], W2[l, k * P:(k + 1) * P, :])

            # ---- attention (processed in head pairs) ----
            # rowsum reciprocals come from the q-major exp's accum_out,
            # stored sparsely at free columns {0,32,64,96} so a PE transpose
            # lands them on 32-aligned partitions for the K=1 broadcast
            # matmuls (f32r can't use tile_position; the broadcast runs bf16).
            OTn = []
            for pr in range(8):
                ot_ps = pso.tile([P, S], F32, tag="o", name=f"otp{l}_{pr}")
                rbs = []
                for hh in range(2):
                    h = 2 * pr + hh
                    ho = (h % 2) * DK
                    q_hT = QT[pr][ho:ho + DK, :]
                    k_hT = KT[pr][ho:ho + DK, :]

                    rs = mk([P, NS], F32, "rs", 2, f"rs{l}_{h}")
                    atn = []
                    for qt in range(NS):
                        pt = pss.tile([P, S], F32, tag="s", name=f"sc{l}_{h}_{qt}")
                        nc.tensor.matmul(pt[:], q_hT[:, qt * P:(qt + 1) * P],
                                         k_hT[:], start=True, stop=True)
                        a = mk([P, S], BF16, "attn", 5, f"at{l}_{h}_{qt}")
                        nc.scalar.activation(a[:], pt[:], AF.Exp, scale=0.125,
                                             accum_out=rs[:, qt:qt + 1])
                        atn.append(a)
                    et = []
                    eth = [mk([P, 2 * S], BF16, "et", 2, f"et{l}_{h}_{half}")
                           for half in range(2)]
                    for half in range(2):
                        pt = psT.tile([P, 2 * S], F32, tag="T", name=f"sT{l}_{h}_{half}")
                        for j in range(2):
                            kt = 2 * half + j
                            nc.tensor.matmul(pt[:, j * S:(j + 1) * S],
                                             k_hT[:, kt * P:(kt + 1) * P],
                                             q_hT[:], start=True, stop=True)
                        nc.scalar.activation(eth[half][:], pt[:], AF.Exp,
                                             scale=0.125)
                        et.append(eth[half][:, 0:S])
                        et.append(eth[half][:, S:2 * S])
                    for kt in range(NS):
                        nc.tensor.matmul(ot_ps[ho:ho + DK, :],
                                         Vt[kt][:, h * DK:(h + 1) * DK],
                                         et[kt][:],
                                         start=(kt == 0), stop=(kt == NS - 1),
                                         tile_position=(0, ho))

                    rcp = mk([P, NS], F32, "rcp", 2, f"rcp{l}_{h}")
                    nc.vector.reciprocal(rcp[:], rs[:])
                    for qt in range(NS):
                        nc.vector.tensor_scalar_mul(atn[qt][:], atn[qt][:],
                                                    rcp[:, qt:qt + 1])
                        nc.sync.dma_start(
                            out_attn[l, h, qt * P:(qt + 1) * P, :], atn[qt][:])
                    # reciprocals -> one [1,512] row (PE transposes), then
                    # broadcast across partitions on the (idle) GpSimd
                    rcpt_ps = psm.tile([P, S], F32, tag="m", name=f"rtp{l}_{h}")
                    for qt in range(NS):
                        nc.tensor.transpose(
                            rcpt_ps[0:1, qt * P:(qt + 1) * P],
                            rcp[:, qt:qt + 1], ident[:])
                    rrow = mk([1, S], F32, "rrow", 2, f"rrow{l}_{h}")
                    nc.scalar.activation(rrow[:], rcpt_ps[0:1, :], AF.Identity)
                    rb_h = mk([P, S], F32, "rb", 3, f"rbs{l}_{h}")
                    nc.gpsimd.partition_broadcast(rb_h[:], rrow[:])
                    rbs.append(rb_h)
                # pair epilogue: OT_norm = OT_unnorm * recipB + bv
                ot = mk([P, S], F32R, "ot", 9, f"ot{l}_{pr}")
                nc.vector.tensor_mul(ot[0:DK, :], ot_ps[0:DK, :],
                                     rbs[0][0:DK, :])
                nc.vector.tensor_mul(ot[DK:P, :], ot_ps[DK:P, :],
                                     rbs[1][DK:P, :])
                nc.vector.tensor_scalar_add(ot[:], ot[:], bv_c[:, pr:pr + 1])
                OTn.append(ot)

            # ---- O projection (+bo) -> y -> LN1 ----
            y1 = []
            for m in range(NM):
                pt = psm.tile([P, S], F32, tag="m", name=f"op{l}_{m}")
                for k in range(NM):
                    nc.tensor.matmul(pt[:], wo_t[k][:, m * P:(m + 1) * P],
                                     OTn[k][:], start=(k == 0),
                                     stop=(k == NM - 1))
                yt = mk([P, S], F32R, "y", 9, f"y1_{l}_{m}")
                nc.scalar.activation(yt[:], pt[:], AF.Identity,
                                     bias=bo_c[:, m:m + 1])
                nc.vector.tensor_add(yt[:], yt[:], xT[m][:])
                y1.append(yt)
            ar, br = lrows[0]
            xT = layer_norm(l, 0, y1, ar, br)

            # ---- FFN ----
            hT = []
            for mf in range(NF):
                pt = psm.tile([P, S], F32, tag="m", name=f"f1p{l}_{mf}")
                for k in range(NM):
                    nc.tensor.matmul(pt[:], w1_t[k][:, mf * P:(mf + 1) * P],
                                     xT[k][:], start=(k == 0),
                                     stop=(k == NM - 1))
                t = mk([P, S], F32R, "h", 5, f"h{l}_{mf}")
                nc.scalar.activation(t[:], pt[:], AF.Relu,
                                     bias=b1_c[:, mf:mf + 1])
                hT.append(t)
            y2 = []
            for m in range(NM):
                pt = psm.tile([P, S], F32, tag="m", name=f"f2p{l}_{m}")
                for kt in range(NF):
                    nc.tensor.matmul(pt[:], w2_t[kt][:, m * P:(m + 1) * P],
                                     hT[kt][:], start=(kt == 0),
                                     stop=(kt == NF - 1))
                yt = mk([P, S], F32R, "y", 9, f"y2_{l}_{m}")
                nc.scalar.activation(yt[:], pt[:], AF.Identity,
                                     bias=b2_c[:, m:m + 1])
                nc.vector.tensor_add(yt[:], yt[:], xT[m][:])
                y2.append(yt)
            ar, br = lrows[1]
            xT = layer_norm(l, 1, y2, ar, br)

        # ---- final transpose back: xT -> out_x [S, D] ----
        for stt in range(NS):
            t = mk([P, D], F32, "t", 2, f"xout{stt}")
            for m in range(NM):
                pt = psm.tile([P, S], F32, tag="m", name=f"tpf{stt}_{m}")
                nc.tensor.transpose(
                    pt[:, 0:P],
                    xT[m][:, stt * P:(stt + 1) * P].bitcast(F32), ident[:])
                nc.vector.tensor_copy(t[:, m * P:(m + 1) * P], pt[:, 0:P])
            nc.sync.dma_start(out_x[stt * P:(stt + 1) * P, :], t[:])

    nc.compile()
    return nc


def _get_nc(nl=NL):
    key = ("nc", nl)
    if key not in _CACHE:
        _CACHE[key] = _build(nl)
    return _CACHE[key]


def kernel(src, Wq, bq, Wk, bk, Wv, bv, Wo, bo,
           ln1_a, ln1_b, ln2_a, ln2_b, W1, b1, W2, b2, nl=NL, trace=False):
    sys.path.insert(0, "/opt/trn_rl_repo")
    from concourse.bass_utils import run_bass_kernel_spmd

    nc = _get_nc(nl)
    if trace:
        try:
            import types
            import antenv
            if 'antenv.axon_hooks' not in sys.modules:
                mod = types.ModuleType('antenv.axon_hooks')
                _h = {}
                mod.set_axon_ntff_profile_hook = lambda h: _h.__setitem__('h', h)
                mod.get_axon_ntff_profile_hook = lambda: _h.get('h')
                sys.modules['antenv.axon_hooks'] = mod
                antenv.axon_hooks = mod
                if '/root/.axon_site' not in sys.path:
                    sys.path.insert(0, '/root/.axon_site')
                from trn_agent_boot.trn_boot import _ntff_profile_via_ctypes
                mod.set_axon_ntff_profile_hook(
                    _ntff_profile_via_ctypes('/opt/axon/libaxon_pjrt.so'))
            import concourse.bass_utils as bu
            bu.upload_artifacts = lambda tmpdir: f"local:{tmpdir}"
        except Exception as e:
            print("trace setup failed:", repr(e))
    f32 = np.float32
    shared = dict(
        Wq=np.ascontiguousarray(Wq, f32), Wk=np.ascontiguousarray(Wk, f32),
        Wv=np.ascontiguousarray(Wv, f32), Wo=np.ascontiguousarray(Wo, f32),
        W1=np.ascontiguousarray(W1, f32), W2=np.ascontiguousarray(W2, f32),
        bq=np.ascontiguousarray(bq, f32), bk=np.ascontiguousarray(bk, f32),
        bv=np.ascontiguousarray(bv, f32), bo=np.ascontiguousarray(bo, f32),
        b1=np.ascontiguousarray(b1, f32), b2=np.ascontiguousarray(b2, f32),
        ln1_a=np.ascontiguousarray(ln1_a, f32),
        ln1_b=np.ascontiguousarray(ln1_b, f32),
        ln2_a=np.ascontiguousarray(ln2_a, f32),
        ln2_b=np.ascontiguousarray(ln2_b, f32),
    )
    in_maps = []
    for b in range(B):
        m = dict(shared)
        m["src"] = np.ascontiguousarray(src[b], f32)
        in_maps.append(m)
    res = run_bass_kernel_spmd(nc, in_maps, core_ids=list(range(B)),
                               trace=trace,
                               tmpdir="/tmp/ktrace" if trace else None)
    x = np.stack([res.results[b]["out_x"] for b in range(B)])
    attn = np.stack([res.results[b]["out_attn"].astype(np.float32)
                     for b in range(B)])
    if trace:
        _CACHE["exec_time_ns"] = res.exec_time_ns
    return (x, attn)
